# revision 3
# baseline (speedup 1.0000x reference)
"""AFNO (Adaptive Fourier Neural Operator) block for Trainium2, 8 NeuronCores.

Sharding: core k owns AFNO block k (96 channels): FFTs are per-channel
(local), the block-diagonal channel MLP only mixes channels within a block,
so there are no collectives; the host concatenates per-core channel slices.

All FFTs are dense band-limited DFT matmuls (the output spectrum is zero
outside the kept band, and softshrink(0)=0, so only the band
[kh 26:232, kw 0:103] is ever computed). The forward H-DFT computes only the
104 unique rows {26..128, 231} and reconstructs the mirrored rows
{129..230} by conjugate symmetry of the real input. The inverse folds the
same symmetry of the inverse-H DFT matrix into a pair-precombine, halving
the W-inverse and H-inverse matmul work.

Band "pos" layout (what the MLP sees; order is arbitrary but must be
consistent between forward collapse / MLP / inverse expand):
  pos = kw*206 + t,  t in [0,104) -> unique kh rows {26..128, 231},
                     t in [104,206) -> mirrored kh' = 230 - (t-104).
"""

import numpy as np
import ml_dtypes

import concourse.bass as bass
import concourse.mybir as mybir
import concourse.tile as tile
from concourse import bacc
from concourse.bass_utils import run_bass_kernel_spmd

H = W = 256
NB = 8          # num_blocks == n_cores
BS = 96         # block_size (channels per block)
B = 2           # batch
NIMG = B * BS   # images per core
KEPT = 103      # kept modes along W
LO, HI = 26, 232
KH = HI - LO          # 206 kept rows along H
NKHU = 104            # unique kh rows {26..128, 231}
NPOS = KEPT * KH      # 21218 band positions
LAM = 0.01            # softshrink lambda
CHUNK = 512
BF16 = ml_dtypes.bfloat16

F32 = mybir.dt.float32
BF = mybir.dt.bfloat16
ALU = mybir.AluOpType
ACT = mybir.ActivationFunctionType


def _build_consts():
    n = np.arange(H, dtype=np.float64)
    kh_u = np.concatenate([np.arange(LO, 129), [231]]).astype(np.float64)  # 104
    # forward H-DFT (band rows), [h, khu]
    ang_h = 2 * np.pi * np.outer(n, kh_u) / H
    ACu = np.cos(ang_h) / 16.0
    ASu = -np.sin(ang_h) / 16.0
    at = np.empty((2, 128, 2 * NKHU), dtype=np.float64)
    for hc in range(2):
        at[hc, :, :NKHU] = ACu[hc * 128:(hc + 1) * 128]
        at[hc, :, NKHU:] = ASu[hc * 128:(hc + 1) * 128]
    # forward W-DFT, [w, kw]
    kw = np.arange(KEPT, dtype=np.float64)
    ang_w = 2 * np.pi * np.outer(n, kw) / W
    FC = np.cos(ang_w) / 16.0
    FS = -np.sin(ang_w) / 16.0
    fc = np.stack([FC[:128], FC[128:]])
    fs = np.stack([FS[:128], FS[128:]])
    # inverse W (irfft with modes 0..102 only; imag of DC ignored), [kw, w]
    alpha = np.where(kw == 0, 1.0, 2.0)
    ang_iw = 2 * np.pi * np.outer(kw, n) / W
    CR = (alpha[:, None] * np.cos(ang_iw)) / 16.0
    CI = (-alpha[:, None] * np.sin(ang_iw)) / 16.0
    CI[0, :] = 0.0
    # inverse H on unique rows, [khu, h]
    bhre = (np.cos(ang_h) / 16.0).T.copy()
    bhim = (np.sin(ang_h) / 16.0).T.copy()
    c = {
        "at": at, "fc": fc, "fs": fs,
        "cr": CR, "ci": CI, "crn": -CR,
        "bhre": bhre, "bhim": bhim,
    }
    return {k: v.astype(BF16) for k, v in c.items()}


_CONSTS = None
_PROGRAM = None


def _get_consts():
    global _CONSTS
    if _CONSTS is None:
        _CONSTS = _build_consts()
    return _CONSTS


def build_program():
    nc = bacc.Bacc(None, target_bir_lowering=False, debug=False)

    x_d = nc.dram_tensor("x", [NIMG, H, W], F32, kind="ExternalInput")
    out_d = nc.dram_tensor("out", [NIMG, H, W], F32, kind="ExternalOutput")
    at_d = nc.dram_tensor("at", [2, 128, 2 * NKHU], BF, kind="ExternalInput")
    fc_d = nc.dram_tensor("fc", [2, 128, KEPT], BF, kind="ExternalInput")
    fs_d = nc.dram_tensor("fs", [2, 128, KEPT], BF, kind="ExternalInput")
    cr_d = nc.dram_tensor("cr", [KEPT, W], BF, kind="ExternalInput")
    ci_d = nc.dram_tensor("ci", [KEPT, W], BF, kind="ExternalInput")
    crn_d = nc.dram_tensor("crn", [KEPT, W], BF, kind="ExternalInput")
    bhre_d = nc.dram_tensor("bhre", [NKHU, H], BF, kind="ExternalInput")
    bhim_d = nc.dram_tensor("bhim", [NKHU, H], BF, kind="ExternalInput")
    w_d = {
        nm: nc.dram_tensor(nm, [BS, BS], BF, kind="ExternalInput")
        for nm in ("w1re", "w1im", "w1imn", "w2re", "w2im", "w2imn")
    }
    b_d = {
        nm: nc.dram_tensor(nm, [BS, 1], F32, kind="ExternalInput")
        for nm in ("b1re", "b1im", "b2re", "b2im")
    }

    with tile.TileContext(nc) as tc:
        with (
            tc.tile_pool(name="consts", bufs=1) as consts,
            tc.tile_pool(name="band", bufs=1) as bandp,
            tc.tile_pool(name="work", bufs=3) as work,
        ):
            at_t = []
            fc_t = []
            fs_t = []
            for hc in range(2):
                t = consts.tile([128, 2 * NKHU], BF, tag=f"at{hc}", name=f"at{hc}")
                nc.sync.dma_start(t[:], at_d[hc, :, :])
                at_t.append(t)
                t = consts.tile([128, KEPT], BF, tag=f"fc{hc}", name=f"fc{hc}")
                nc.sync.dma_start(t[:], fc_d[hc, :, :])
                fc_t.append(t)
                t = consts.tile([128, KEPT], BF, tag=f"fs{hc}", name=f"fs{hc}")
                nc.sync.dma_start(t[:], fs_d[hc, :, :])
                fs_t.append(t)
            cr_t = consts.tile([KEPT, W], BF, tag="cr")
            nc.sync.dma_start(cr_t[:], cr_d[:, :])
            ci_t = consts.tile([KEPT, W], BF, tag="ci")
            nc.sync.dma_start(ci_t[:], ci_d[:, :])
            crn_t = consts.tile([KEPT, W], BF, tag="crn")
            nc.sync.dma_start(crn_t[:], crn_d[:, :])
            bhre_t = consts.tile([NKHU, H], BF, tag="bhre")
            nc.sync.dma_start(bhre_t[:], bhre_d[:, :])
            bhim_t = consts.tile([NKHU, H], BF, tag="bhim")
            nc.sync.dma_start(bhim_t[:], bhim_d[:, :])
            w_t = {}
            for nm in w_d:
                w_t[nm] = consts.tile([BS, BS], BF, tag=nm, name=nm)
                nc.sync.dma_start(w_t[nm][:], w_d[nm][:, :])
            b_t = {}
            for nm in b_d:
                b_t[nm] = consts.tile([BS, 1], F32, tag=nm, name=nm)
                nc.sync.dma_start(b_t[nm][:], b_d[nm][:, :])

            for b in range(B):
                band_re = bandp.tile([BS, NPOS], BF, tag="band_re")
                band_im = bandp.tile([BS, NPOS], BF, tag="band_im")

                # ---------------- forward ----------------
                with tc.tile_pool(name=f"psF{b}", bufs=2, space="PSUM") as psF:
                    for c in range(BS):
                        nimg = b * BS + c
                        xt = work.tile([128, 512], BF, tag="xt")
                        for hc in range(2):
                            nc.gpsimd.dma_start(
                                xt[:, hc * 256:(hc + 1) * 256],
                                x_d[nimg, hc * 128:(hc + 1) * 128, :],
                            )
                        ps1 = psF.tile([128, 416], F32, tag="ps1")
                        for wc in range(2):
                            for hc in range(2):
                                nc.tensor.matmul(
                                    ps1[:, wc * 208:wc * 208 + 208],
                                    lhsT=xt[:, hc * 256 + wc * 128: hc * 256 + wc * 128 + 128],
                                    rhs=at_t[hc][:],
                                    start=(hc == 0), stop=(hc == 1),
                                )
                        z1 = work.tile([128, 416], BF, tag="z1")
                        nc.vector.tensor_copy(z1[:], ps1[:])
                        # P1|P2|P3|P4 = FC@Z1re | FS@Z1im | FS@Z1re | FC@Z1im
                        # each P region is a consecutive start->stop group:
                        # start=True clears has_written for the WHOLE bank, so
                        # groups must not interleave within one PSUM bank.
                        pps = psF.tile([KEPT, 416], F32, tag="pps")
                        for sl_p, wtab, zoff in (
                            (slice(0, 104), fc_t, 0),        # P1 = FC @ Z1re
                            (slice(104, 208), fs_t, 104),    # P2 = FS @ Z1im
                            (slice(208, 312), fs_t, 0),      # P3 = FS @ Z1re
                            (slice(312, 416), fc_t, 104),    # P4 = FC @ Z1im
                        ):
                            for wc in range(2):
                                nc.tensor.matmul(
                                    pps[:, sl_p],
                                    lhsT=wtab[wc][:],
                                    rhs=z1[:, wc * 208 + zoff: wc * 208 + zoff + 104],
                                    start=(wc == 0), stop=(wc == 1),
                                )
                        p_s = work.tile([KEPT, 416], BF, tag="p_s")
                        nc.scalar.copy(p_s[:], pps[:])
                        # plane: [re_u 104 | re_m 102 | im_u 104 | im_m 102]
                        plane = work.tile([KEPT, 412], BF, tag="plane")
                        nc.vector.tensor_tensor(plane[:, 0:104], p_s[:, 0:104], p_s[:, 104:208], ALU.subtract)
                        nc.vector.tensor_tensor(plane[:, 104:206], p_s[:, 0:102], p_s[:, 104:206], ALU.add)
                        nc.vector.tensor_tensor(plane[:, 206:310], p_s[:, 208:312], p_s[:, 312:416], ALU.add)
                        nc.vector.tensor_tensor(plane[:, 310:412], p_s[:, 208:310], p_s[:, 312:414], ALU.subtract)
                        nc.sync.dma_start(band_re[c:c + 1, :], plane[:, 0:206])
                        nc.sync.dma_start(band_im[c:c + 1, :], plane[:, 206:412])

                # ---------------- block MLP ----------------
                with tc.tile_pool(name=f"psM{b}", bufs=2, space="PSUM") as psM:
                    for st0 in range(0, NPOS, CHUNK):
                        wj = min(CHUNK, NPOS - st0)
                        sl = slice(st0, st0 + wj)
                        are = band_re[:, sl]
                        aim = band_im[:, sl]
                        o1re_ps = psM.tile([BS, CHUNK], F32, tag="o1re")
                        o1im_ps = psM.tile([BS, CHUNK], F32, tag="o1im")
                        nc.tensor.matmul(o1im_ps[:, :wj], lhsT=w_t["w1im"][:], rhs=are, start=True, stop=False)
                        nc.tensor.matmul(o1re_ps[:, :wj], lhsT=w_t["w1re"][:], rhs=are, start=True, stop=False)
                        nc.tensor.matmul(o1im_ps[:, :wj], lhsT=w_t["w1re"][:], rhs=aim, start=False, stop=True)
                        nc.tensor.matmul(o1re_ps[:, :wj], lhsT=w_t["w1imn"][:], rhs=aim, start=False, stop=True)
                        o1re_s = work.tile([BS, CHUNK], BF, tag="o1re_s")
                        o1im_s = work.tile([BS, CHUNK], BF, tag="o1im_s")
                        nc.scalar.activation(o1re_s[:, :wj], o1re_ps[:, :wj], ACT.Relu, bias=b_t["b1re"][:])
                        nc.scalar.activation(o1im_s[:, :wj], o1im_ps[:, :wj], ACT.Relu, bias=b_t["b1im"][:])
                        o2re_ps = psM.tile([BS, CHUNK], F32, tag="o2re")
                        o2im_ps = psM.tile([BS, CHUNK], F32, tag="o2im")
                        nc.tensor.matmul(o2im_ps[:, :wj], lhsT=w_t["w2im"][:], rhs=o1re_s[:, :wj], start=True, stop=False)
                        nc.tensor.matmul(o2re_ps[:, :wj], lhsT=w_t["w2re"][:], rhs=o1re_s[:, :wj], start=True, stop=False)
                        nc.tensor.matmul(o2im_ps[:, :wj], lhsT=w_t["w2re"][:], rhs=o1im_s[:, :wj], start=False, stop=True)
                        nc.tensor.matmul(o2re_ps[:, :wj], lhsT=w_t["w2imn"][:], rhs=o1im_s[:, :wj], start=False, stop=True)
                        # y = o2 + b2 ; softshrink(y) = y - clamp(y, -lam, lam)
                        yre = work.tile([BS, CHUNK], F32, tag="yre")
                        yim = work.tile([BS, CHUNK], F32, tag="yim")
                        nc.scalar.activation(yre[:, :wj], o2re_ps[:, :wj], ACT.Identity, bias=b_t["b2re"][:])
                        nc.scalar.activation(yim[:, :wj], o2im_ps[:, :wj], ACT.Identity, bias=b_t["b2im"][:])
                        tre = work.tile([BS, CHUNK], F32, tag="tre")
                        tim = work.tile([BS, CHUNK], F32, tag="tim")
                        nc.vector.tensor_scalar(tre[:, :wj], yre[:, :wj], -LAM, LAM, ALU.max, ALU.min)
                        nc.vector.tensor_scalar(tim[:, :wj], yim[:, :wj], -LAM, LAM, ALU.max, ALU.min)
                        nc.vector.tensor_tensor(band_re[:, sl], yre[:, :wj], tre[:, :wj], ALU.subtract)
                        nc.vector.tensor_tensor(band_im[:, sl], yim[:, :wj], tim[:, :wj], ALU.subtract)

                # ---------------- inverse ----------------
                with tc.tile_pool(name=f"psI{b}", bufs=2, space="PSUM") as psI:
                    for c in range(BS):
                        nimg = b * BS + c
                        plane = work.tile([KEPT, 412], BF, tag="plane_i")
                        nc.sync.dma_start(plane[:, 0:206], band_re[c:c + 1, :])
                        nc.sync.dma_start(plane[:, 206:412], band_im[c:c + 1, :])
                        # pre: [re_sym 104 | re_asym 104 | im_sym 104 | im_asym 104]
                        pre = work.tile([KEPT, 416], BF, tag="pre")
                        nc.vector.tensor_tensor(pre[:, 0:102], plane[:, 0:102], plane[:, 104:206], ALU.add)
                        nc.vector.tensor_copy(pre[:, 102:104], plane[:, 102:104])
                        nc.vector.tensor_tensor(pre[:, 104:206], plane[:, 0:102], plane[:, 104:206], ALU.subtract)
                        nc.vector.tensor_copy(pre[:, 206:208], plane[:, 102:104])
                        nc.vector.tensor_tensor(pre[:, 208:310], plane[:, 206:308], plane[:, 310:412], ALU.add)
                        nc.vector.tensor_copy(pre[:, 310:312], plane[:, 308:310])
                        nc.vector.tensor_tensor(pre[:, 312:414], plane[:, 206:308], plane[:, 310:412], ALU.subtract)
                        nc.vector.tensor_copy(pre[:, 414:416], plane[:, 308:310])
                        # W-inverse: Gre|Gim [104, 256] each
                        gps = psI.tile([NKHU, 512], F32, tag="gps")
                        nc.tensor.matmul(gps[:, 0:256], lhsT=pre[:, 0:104], rhs=cr_t[:], start=True, stop=False)
                        nc.tensor.matmul(gps[:, 0:256], lhsT=pre[:, 208:312], rhs=ci_t[:], start=False, stop=True)
                        nc.tensor.matmul(gps[:, 256:512], lhsT=pre[:, 104:208], rhs=ci_t[:], start=True, stop=False)
                        nc.tensor.matmul(gps[:, 256:512], lhsT=pre[:, 312:416], rhs=crn_t[:], start=False, stop=True)
                        g_s = work.tile([NKHU, 512], BF, tag="g_s")
                        nc.scalar.copy(g_s[:], gps[:])
                        # H-inverse (real part only)
                        ops = psI.tile([128, 512], F32, tag="ops")
                        for hc in range(2):
                            nc.tensor.matmul(
                                ops[:, hc * 256:hc * 256 + 256],
                                lhsT=bhre_t[:, hc * 128:hc * 128 + 128],
                                rhs=g_s[:, 0:256], start=True, stop=False,
                            )
                            nc.tensor.matmul(
                                ops[:, hc * 256:hc * 256 + 256],
                                lhsT=bhim_t[:, hc * 128:hc * 128 + 128],
                                rhs=g_s[:, 256:512], start=False, stop=True,
                            )
                        xr = work.tile([128, 512], F32, tag="xr")
                        for hc in range(2):
                            nc.sync.dma_start(
                                xr[:, hc * 256:(hc + 1) * 256],
                                x_d[nimg, hc * 128:(hc + 1) * 128, :],
                            )
                        outs = work.tile([128, 512], F32, tag="outs")
                        nc.vector.tensor_tensor(outs[:, 0:256], ops[:, 0:256], xr[:, 0:256], ALU.add)
                        nc.vector.tensor_tensor(outs[:, 256:512], ops[:, 256:512], xr[:, 256:512], ALU.add)
                        for hc in range(2):
                            nc.sync.dma_start(
                                out_d[nimg, hc * 128:(hc + 1) * 128, :],
                                outs[:, hc * 256:(hc + 1) * 256],
                            )

    nc.finalize()
    return nc


def _get_program():
    global _PROGRAM
    if _PROGRAM is None:
        _PROGRAM = build_program()
    return _PROGRAM


def make_in_maps(x, w1, b1, w2, b2):
    consts = _get_consts()
    in_maps = []
    for k in range(NB):
        m = dict(consts)
        m["x"] = np.ascontiguousarray(
            x[:, k * BS:(k + 1) * BS].reshape(NIMG, H, W)
        )
        m["w1re"] = np.ascontiguousarray(w1[k, :, :, 0]).astype(BF16)
        m["w1im"] = np.ascontiguousarray(w1[k, :, :, 1]).astype(BF16)
        m["w1imn"] = np.ascontiguousarray(-w1[k, :, :, 1]).astype(BF16)
        m["w2re"] = np.ascontiguousarray(w2[k, :, :, 0]).astype(BF16)
        m["w2im"] = np.ascontiguousarray(w2[k, :, :, 1]).astype(BF16)
        m["w2imn"] = np.ascontiguousarray(-w2[k, :, :, 1]).astype(BF16)
        m["b1re"] = np.ascontiguousarray(b1[k, :, 0, 0, 0]).reshape(BS, 1).astype(np.float32)
        m["b1im"] = np.ascontiguousarray(b1[k, :, 0, 0, 1]).reshape(BS, 1).astype(np.float32)
        m["b2re"] = np.ascontiguousarray(b2[k, :, 0, 0, 0]).reshape(BS, 1).astype(np.float32)
        m["b2im"] = np.ascontiguousarray(b2[k, :, 0, 0, 1]).reshape(BS, 1).astype(np.float32)
        in_maps.append(m)
    return in_maps


def kernel(x, w1, b1, w2, b2, trace=False, tmpdir=None):
    x = np.asarray(x, dtype=np.float32)
    w1 = np.asarray(w1, dtype=np.float32)
    b1 = np.asarray(b1, dtype=np.float32)
    w2 = np.asarray(w2, dtype=np.float32)
    b2 = np.asarray(b2, dtype=np.float32)
    nc = _get_program()
    in_maps = make_in_maps(x, w1, b1, w2, b2)
    res = run_bass_kernel_spmd(
        nc, in_maps, list(range(NB)), trace=trace, tmpdir=tmpdir
    )
    out = np.empty((B, NB * BS, H, W), dtype=np.float32)
    for k in range(NB):
        out[:, k * BS:(k + 1) * BS] = res.results[k]["out"].reshape(B, BS, H, W)
    if trace:
        kernel.last_exec_time_ns = res.exec_time_ns
    return out
